# revision 15
# baseline (speedup 1.0000x reference)
"""Trainium2 Bass kernel for a single-layer attention decoder.

Model (see problem reference): B=8 batches, T=S=512, D=512, V=32000.
  x = embed(trg) + PE
  x = LN(x + SelfAttn(x, causal+pad mask))
  x = LN(x + CrossAttn(x, enc, src-length mask))
  x = LN(x + FFN(x))            # D -> 4D -> D, relu
  logits = x @ fcw + fcb        # [B, T, V]

Distribution over 8 NeuronCores:
  - decoder body: data-parallel over batch (1 batch row per core)
  - AllGather of the (transposed, LN3-normalized) body output in four
    t-quarters (bf16, 0.125 MB/core each), each triggered as the FFN tail
    produces its quarter so the collectives overlap the body tail / fc head
  - fc projection: tensor-parallel over vocab (each core: all batches x V/8
    cols), logits stored bf16 and upcast on the host

Key algebraic folds (host side): 1/sqrt(D) and Wq^T Wk into a single
score matrix M (scores = x M x^T, valid because the q/k biases are zero),
LN3 gain into fcw, softmax 1/sum deferred into the attention-output scale.
"""

import math
import sys
import types

import numpy as np
import ml_dtypes

B, T, S, D, V = 8, 512, 512, 512, 32000
N_CORES = 8
VS = V // N_CORES          # 4000 vocab cols per core
P = 128
KT = D // P                # 4 contraction tiles
TT = T // P                # 4 row tiles
JT = (4 * D) // P          # 16 ffn hidden tiles
VCH = 500                  # fc free-dim chunk
NVCH = VS // VCH           # 8 chunks
TH = T // 2                # 256-col t-half for the FFN tail
EPS = 1e-5
NEG = -1e9

BF = ml_dtypes.bfloat16

# toggles for the test harness
TRACE = False
TRACE_DIR = None
MODE = "full"
LAST_EXEC_NS = None
LAST_RESULTS = None

# Transpose 128x128 blocks on the DMA XBAR instead of the PE array.
# Measured: the XBAR path floods the DMA queues (~300 packets per block)
# and stalls the pipeline — keep the PE-array transposes.
DMA_TRANSPOSE = False

_cache = {}


def _sinusoidal_pe(length, d):
    pos = np.arange(length)[:, None].astype(np.float32)
    div = np.exp(np.arange(0, d, 2).astype(np.float32) * (-np.log(10000.0) / d))
    pe = np.zeros((length, d), dtype=np.float32)
    pe[:, 0::2] = np.sin(pos * div)
    pe[:, 1::2] = np.cos(pos * div)
    return pe


def _bf(x):
    return np.ascontiguousarray(np.asarray(x, dtype=np.float32).astype(BF))


def _f32(x):
    return np.ascontiguousarray(np.asarray(x, dtype=np.float32))


def _install_ntff_hook():
    """Register the NTFF profiling hook so trace=True works under axon."""
    if "antenv.axon_hooks" in sys.modules:
        return
    try:
        from trn_agent_boot.trn_boot import _ntff_profile_via_ctypes
        from concourse import bass_utils

        hook = _ntff_profile_via_ctypes("/opt/axon/libaxon_pjrt.so")
        mod = types.ModuleType("antenv.axon_hooks")
        mod.get_axon_ntff_profile_hook = lambda: hook
        mod.set_axon_ntff_profile_hook = lambda h: None
        sys.modules["antenv.axon_hooks"] = mod
        bass_utils.upload_artifacts = lambda tmpdir: "local://" + tmpdir
    except Exception:
        pass


def _build(cfg, mode="full"):
    """Build + compile the per-core SPMD graph. cfg keys:
    exact_mask, qk1, qk2, b1, gb1, gb2, b2, fcb.
    mode: "full" | "body" (stop after LN3, dump xn3) | "ag" (body + split
    AllGather, dump the gathered quarters re-packed as [B*D, T])."""
    from concourse import bacc, tile
    import concourse.mybir as mybir

    F32 = mybir.dt.float32
    BF16 = mybir.dt.bfloat16
    Alu = mybir.AluOpType
    Act = mybir.ActivationFunctionType

    nc = bacc.Bacc("TRN2", target_bir_lowering=False, debug=False)

    x0_d = nc.dram_tensor("x0", [T, D], BF16, kind="ExternalInput")
    x0T_d = nc.dram_tensor("x0T", [D, T], BF16, kind="ExternalInput")
    encT_d = nc.dram_tensor("encT", [D, S], BF16, kind="ExternalInput")
    sbias_d = nc.dram_tensor("self_bias", [T, S], BF16, kind="ExternalInput")
    cbias_d = nc.dram_tensor("cross_bias", [P, S], BF16, kind="ExternalInput")
    wnames = ["wv1", "wv2"]
    wnames += (["wf1"] if cfg["qk1"] else ["wq1", "wk1"])
    wnames += (["wf2"] if cfg["qk2"] else ["wq2", "wk2"])
    w_d = {nm: nc.dram_tensor(nm, [D, D], BF16, kind="ExternalInput") for nm in wnames}
    w1_d = nc.dram_tensor("w1", [D, 4 * D], BF16, kind="ExternalInput")
    w2_d = nc.dram_tensor("w2", [4 * D, D], BF16, kind="ExternalInput")
    fcw_d = nc.dram_tensor("fcw", [D, VS], BF16, kind="ExternalInput")
    id_d = None
    if not DMA_TRANSPOSE:
        id_d = nc.dram_tensor("ident", [P, P], BF16, kind="ExternalInput")
    # packed per-partition biases: cols 0-3 bq1, 4-7 bk1, 8-11 bq2, 12-15 bk2, 16-31 b1
    biasp_d = nc.dram_tensor("bias_p", [P, 32], F32, kind="ExternalInput")
    # replicated per-free biases: [:,0,:]=bv1, [:,1,:]=bv2
    bvrep_d = nc.dram_tensor("bias_v", [P, 2, D], BF16, kind="ExternalInput")
    smask_d = None
    if cfg["exact_mask"]:
        smask_d = nc.dram_tensor("self_m01", [T, S], BF16, kind="ExternalInput")
    gb_d = {}
    for key, shape in (("gb1", [P, 2, D]), ("gb2", [P, 2, D]),
                       ("b2", [P, D]), ("fcb", [P, VS])):
        if cfg[key]:
            gb_d[key] = nc.dram_tensor(key + "_t", shape, BF16, kind="ExternalInput")
    if mode.startswith("body"):
        out_d = nc.dram_tensor("out", [T, D], BF16, kind="ExternalOutput")
    elif mode == "ag":
        out_d = nc.dram_tensor("out", [B * D, T], BF16, kind="ExternalOutput")
    else:
        out_d = nc.dram_tensor("out", [B, T, VS], BF16, kind="ExternalOutput")

    with tile.TileContext(nc) as tc:
        with (
            tc.tile_pool(name="wp", bufs=1) as wp,
            tc.tile_pool(name="big", bufs=10) as big,
            tc.tile_pool(name="hp", bufs=1) as hp,
            tc.tile_pool(name="wk", bufs=3) as wk,
            tc.tile_pool(name="st", bufs=10) as st,
            tc.tile_pool(name="fcx", bufs=3) as fcx,
            tc.tile_pool(name="ps", bufs=6, space="PSUM") as ps_pool,
            tc.tile_pool(name="pst", bufs=2, space="PSUM") as pst_pool,
            tc.tile_pool(name="dr", bufs=1, space="DRAM") as dr,
        ):
            def load3(handle, ktiles, free, dtype, name, split=False):
                t_ = wp.tile([P, ktiles, free], dtype, tag=name, name=name)
                src = handle[:, :].rearrange("(k p) f -> p k f", p=P)
                if split:
                    for k in range(ktiles):
                        nc.sync.dma_start(t_[:, k, :], src[:, k, :])
                else:
                    nc.sync.dma_start(t_[:], src)
                return t_

            def emit_prime():
                # Prime the CC stream with a tiny AllGather so the ncfw /
                # descriptor path is warm before the real gathers fire
                # (cuts the first real gather's trigger delay ~11.5us -> ~1us).
                prime_sb = wp.tile([P, 16], BF16, tag="prime", name="prime")
                nc.vector.memset(prime_sb[:], 0.0)
                prime_in = dr.tile([16, 16], BF16, name="prime_in",
                                   tag="prime_in")
                nc.sync.dma_start(prime_in[:, :], prime_sb[0:16, 0:16])
                prime_out = dr.tile([16 * N_CORES, 16], BF16,
                                    addr_space="Shared",
                                    name="prime_out", tag="prime_out")
                nc.gpsimd.collective_compute(
                    "AllGather",
                    Alu.bypass,
                    replica_groups=[list(range(N_CORES))],
                    ins=[prime_in[:, :].opt()],
                    outs=[prime_out[:, :].opt()],
                )

            collective = mode == "full" or mode == "ag"
            body_sb = {}
            # Loads are emitted in first-use order so the first matmul can
            # start as early as possible; fcw (the largest tensor,
            # fc-phase-only) goes last.
            body_sb["x0T"] = load3(x0T_d, KT, T, BF16, "x0T", split=True)
            nm0 = "wf1" if cfg["qk1"] else "wq1"
            body_sb[nm0] = load3(w_d[nm0], KT, D, BF16, nm0, split=True)
            biasp_sb = wp.tile([P, 32], F32, tag="biasp", name="biasp")
            nc.sync.dma_start(biasp_sb[:], biasp_d[:, :])
            if not cfg["qk1"]:
                body_sb["wk1"] = load3(w_d["wk1"], KT, D, BF16, "wk1")
            body_sb["sbias"] = load3(sbias_d, TT, S, BF16, "sbias")
            if collective:
                emit_prime()
            smask_sb = None
            if cfg["exact_mask"]:
                smask_sb = load3(smask_d, TT, S, BF16, "smask")
            body_sb["wv1"] = load3(w_d["wv1"], KT, D, BF16, "wv1")
            bvrep_sb = wp.tile([P, 2, D], BF16, tag="bvrep", name="bvrep")
            nc.sync.dma_start(bvrep_sb[:], bvrep_d[:, :, :])
            body_sb["encT"] = load3(encT_d, KT, S, BF16, "encT")
            nm2 = "wf2" if cfg["qk2"] else "wk2"
            body_sb[nm2] = load3(w_d[nm2], KT, D, BF16, nm2)
            body_sb["wv2"] = load3(w_d["wv2"], KT, D, BF16, "wv2")
            ident_sb = None
            if not DMA_TRANSPOSE:
                ident_sb = wp.tile([P, P], BF16, tag="ident", name="ident")
                nc.sync.dma_start(ident_sb[:], id_d[:, :])
            body_sb["x0"] = load3(x0_d, TT, D, BF16, "x0")
            if not cfg["qk2"]:
                body_sb["wq2"] = load3(w_d["wq2"], KT, D, BF16, "wq2")
            cbias_sb = wp.tile([P, S], BF16, tag="cbias", name="cbias")
            nc.sync.dma_start(cbias_sb[:], cbias_d[:, :])
            body_sb["w1"] = load3(w1_d, KT, 4 * D, BF16, "w1")
            body_sb["w2"] = load3(w2_d, JT, D, BF16, "w2")
            eps_sb = wp.tile([P, 1], F32, tag="eps", name="eps")
            nc.vector.memset(eps_sb[:], EPS)
            fcw_sb = load3(fcw_d, KT, VS, BF16, "fcw")
            gb_sb = {}
            for key, t_ in gb_d.items():
                sh = [P, 2, D] if key in ("gb1", "gb2") else (
                    [P, D] if key == "b2" else [P, VS])
                gb_sb[key] = wp.tile(sh, BF16, tag=key, name=key)
                nc.sync.dma_start(gb_sb[key][:], t_[(slice(None),) * len(sh)])

            def mm_accum(psum, lhsT_list, rhs_list):
                n = len(lhsT_list)
                for i, (l_, r_) in enumerate(zip(lhsT_list, rhs_list)):
                    nc.tensor.matmul(psum, l_, r_, start=(i == 0), stop=(i == n - 1))

            def project_T(wtile, xtile, name, bcol):
                # out[:, m, :] over d'-tiles; out[d', t] = sum_d w[d, d'] * x[d, t]
                o_ = big.tile([P, KT, T], BF16, tag="big", name=name)
                for m in range(KT):
                    psum = ps_pool.tile([P, T], F32, tag="ps", name="ps")
                    mm_accum(
                        psum[:],
                        [wtile[:, k, m * P:(m + 1) * P] for k in range(KT)],
                        [xtile[:, k, :] for k in range(KT)],
                    )
                    nc.vector.tensor_scalar_add(
                        o_[:, m, :], psum[:], biasp_sb[:, bcol + m:bcol + m + 1]
                    )
                return o_

            def project_V(wtile, xtile, name, bv_idx):
                # out[s, d'] = sum_d x[d, s] * w[d, d']
                o_ = big.tile([P, TT, D], BF16, tag="big", name=name)
                for m in range(TT):
                    psum = ps_pool.tile([P, D], F32, tag="ps", name="ps")
                    mm_accum(
                        psum[:],
                        [xtile[:, k, m * P:(m + 1) * P] for k in range(KT)],
                        [wtile[:, k, :] for k in range(KT)],
                    )
                    nc.vector.tensor_add(o_[:, m, :], psum[:], bvrep_sb[:, bv_idx, :])
                return o_

            def transpose_blocks(dst, src, blocks):
                # dst[:, b, a*P:(a+1)*P] = src[:, a, b*P:(b+1)*P].T per block
                for a_, b_ in blocks:
                    if DMA_TRANSPOSE:
                        nc.sync.dma_start(
                            dst[:, b_, a_ * P:(a_ + 1) * P],
                            src[:, a_, b_ * P:(b_ + 1) * P],
                            transpose=True,
                        )
                    else:
                        pst = pst_pool.tile([P, P], BF16, tag="pst", name="pst")
                        nc.tensor.transpose(
                            pst[:], src[:, a_, b_ * P:(b_ + 1) * P], ident_sb[:]
                        )
                        nc.scalar.copy(dst[:, b_, a_ * P:(a_ + 1) * P], pst[:])

            def transpose512(src, nPart, nFree, name):
                dst = big.tile([P, nFree, nPart * P], BF16, tag="big", name=name)
                transpose_blocks(
                    dst, src, [(a_, b_) for a_ in range(nPart) for b_ in range(nFree)]
                )
                return dst

            def softmax_tile(ps_scores, bias_ap, m01_ap, attn_ap):
                """exp + row-sum; the 1/sum scale is deferred to the
                attention-output stage (returned as a [P,1] reciprocal)."""
                masked = wk.tile([P, S], F32, tag="masked", name="masked")
                if m01_ap is not None:
                    tmp = wk.tile([P, S], F32, tag="masktmp", name="masktmp")
                    nc.vector.tensor_mul(tmp[:], ps_scores, m01_ap)
                    src = tmp[:]
                else:
                    src = ps_scores
                den = st.tile([P, 1], F32, tag="den", name="den")
                nc.vector.tensor_add(masked[:], src, bias_ap)
                if cfg["exact_mask"]:
                    # a row can be fully masked (all -1e9): subtract the max
                    # so exp gives the reference's uniform-attention row
                    negmx = st.tile([P, 1], F32, tag="negmx", name="negmx")
                    nc.vector.reduce_max(
                        negmx[:], masked[:], axis=mybir.AxisListType.X, negate=True
                    )
                    nc.scalar.activation(
                        attn_ap, masked[:], Act.Exp, bias=negmx[:], scale=1.0
                    )
                else:
                    # scores are O(1) and -1e9 underflows exp to exactly 0
                    nc.scalar.activation(attn_ap, masked[:], Act.Exp)
                nc.vector.reduce_sum(den[:], attn_ap, axis=mybir.AxisListType.X)
                rden = st.tile([P, 1], F32, tag="rden", name="rden")
                nc.vector.reciprocal(rden[:], den[:])
                return rden

            def layer_norm(ps_in, scale_ap, res_ap, xn_ap, gb_key):
                """xn = LN(ps_in * scale + res) via E[x^2]-mean^2, with the
                centering+scaling applied in one fused tensor_scalar."""
                xpre = wk.tile([P, D], F32, tag="xpre", name="xpre")
                if scale_ap is not None:
                    xs = wk.tile([P, D], F32, tag="xs", name="xs")
                    nc.vector.tensor_scalar_mul(xs[:], ps_in, scale_ap)
                    nc.vector.tensor_add(xpre[:], xs[:], res_ap)
                else:
                    nc.vector.tensor_add(xpre[:], ps_in, res_ap)
                rsum = st.tile([P, 1], F32, tag="rsum", name="rsum")
                nc.vector.reduce_sum(rsum[:], xpre[:], axis=mybir.AxisListType.X)
                sq = wk.tile([P, D], F32, tag="sq", name="sq")
                nc.vector.tensor_mul(sq[:], xpre[:], xpre[:])
                sqs = st.tile([P, 1], F32, tag="sqs", name="sqs")
                nc.vector.reduce_sum(sqs[:], sq[:], axis=mybir.AxisListType.X)
                negmean = st.tile([P, 1], F32, tag="negmean", name="negmean")
                nc.vector.tensor_scalar_mul(negmean[:], rsum[:], -1.0 / D)
                m2 = st.tile([P, 1], F32, tag="m2", name="m2")
                nc.vector.tensor_mul(m2[:], negmean[:], negmean[:])
                ex2 = st.tile([P, 1], F32, tag="ex2", name="ex2")
                nc.vector.tensor_scalar_mul(ex2[:], sqs[:], 1.0 / D)
                var = st.tile([P, 1], F32, tag="var", name="var")
                nc.vector.tensor_sub(var[:], ex2[:], m2[:])
                std = st.tile([P, 1], F32, tag="std", name="std")
                nc.scalar.activation(std[:], var[:], Act.Sqrt, bias=eps_sb[:])
                rstd = st.tile([P, 1], F32, tag="rstd", name="rstd")
                nc.vector.reciprocal(rstd[:], std[:])
                nc.vector.tensor_scalar(
                    xn_ap, xpre[:], negmean[:], rstd[:], op0=Alu.add, op1=Alu.mult
                )
                if gb_key is not None and cfg[gb_key]:
                    g_ = gb_sb[gb_key]
                    nc.any.tensor_mul(xn_ap, xn_ap, g_[:, 0, :])
                    nc.any.tensor_add(xn_ap, xn_ap, g_[:, 1, :])

            def attention_scores(xT, ptile, bias_sb, per_m_bias, mask_sb, name):
                """attn[t, s] = exp(x^T . p + bias); returns (attn, rdens)."""
                attn = big.tile([P, TT, S], BF16, tag="big", name=name)
                rdens = []
                for m in range(TT):
                    pss = ps_pool.tile([P, S], F32, tag="ps", name="ps")
                    mm_accum(
                        pss[:],
                        [xT[:, k, m * P:(m + 1) * P] for k in range(KT)],
                        [ptile[:, k, :] for k in range(KT)],
                    )
                    rdens.append(softmax_tile(
                        pss[:],
                        bias_sb[:, m, :] if per_m_bias else bias_sb[:, :],
                        mask_sb[:, m, :] if mask_sb is not None else None,
                        attn[:, m, :],
                    ))
                return attn, rdens

            def attention_out(attnT, vtile, rdens, res, name, gb_key):
                x_ = big.tile([P, TT, D], BF16, tag="big", name=name)
                for m in range(TT):
                    pso = ps_pool.tile([P, D], F32, tag="ps", name="ps")
                    mm_accum(
                        pso[:],
                        [attnT[:, s_, m * P:(m + 1) * P] for s_ in range(TT)],
                        [vtile[:, s_, :] for s_ in range(TT)],
                    )
                    layer_norm(pso[:], rdens[m][:], res[:, m, :], x_[:, m, :], gb_key)
                return x_

            def emit_body(after_quarter=None):
                upto = mode.split(":")[1] if ":" in mode else None
                # self attention scores: with the q/k fold, scores = x (M x)^T
                if cfg["qk1"]:
                    p1t = project_T(body_sb["wf1"], body_sb["x0T"], "p1t", 4)
                else:
                    qt = project_T(body_sb["wq1"], body_sb["x0T"], "qt", 0)
                    kt1 = project_T(body_sb["wk1"], body_sb["x0T"], "kt1", 4)
                if upto == "qkv":
                    return p1t if cfg["qk1"] else qt
                attn1, rd1 = attention_scores(
                    body_sb["x0T"] if cfg["qk1"] else qt,
                    p1t if cfg["qk1"] else kt1,
                    body_sb["sbias"], True, smask_sb, "attn1",
                )
                # independent projections interleaved so every softmax / LN
                # window has a 16-matmul PE filler
                v1 = project_V(body_sb["wv1"], body_sb["x0T"], "v1", 0)
                if upto == "sm":
                    return attn1
                attn1T = transpose512(attn1, TT, TT, "attn1T")
                if cfg["qk2"]:
                    p2t = project_T(body_sb["wf2"], body_sb["encT"], "p2t", 12)
                else:
                    k2t = project_T(body_sb["wk2"], body_sb["encT"], "k2t", 12)
                if upto == "tr":
                    return attn1T
                x1 = attention_out(attn1T, v1, rd1, body_sb["x0"], "x1", "gb1")
                if upto == "x1":
                    return x1
                x1T = transpose512(x1, TT, KT, "x1T")

                # cross attention
                if cfg["qk2"]:
                    q_xT = x1T
                    q_p = p2t
                else:
                    q2t = project_T(body_sb["wq2"], x1T, "q2t", 8)
                    q_xT = q2t
                    q_p = k2t
                attn2, rd2 = attention_scores(
                    q_xT, q_p, cbias_sb, False, None, "attn2",
                )
                v2 = project_V(body_sb["wv2"], body_sb["encT"], "v2", 1)
                attn2T = transpose512(attn2, TT, TT, "attn2T")
                x2 = attention_out(attn2T, v2, rd2, x1, "x2", "gb2")
                x2T = transpose512(x2, TT, KT, "x2T")

                # FFN, t-quarter at a time; LN3 + transpose per quarter so
                # each AllGather fires as soon as its quarter is ready.
                if cfg["b2"]:
                    x2r = big.tile([P, TT, D], BF16, tag="big", name="x2r")
                    for m in range(TT):
                        nc.any.tensor_add(
                            x2r[:, m, :], x2[:, m, :], gb_sb["b2"][:, :]
                        )
                else:
                    x2r = x2
                hT = hp.tile([P, JT, T], BF16, tag="hT", name="hT")
                xn3 = big.tile([P, TT, D], BF16, tag="big", name="xn3")
                x3T = big.tile([P, KT, T], BF16, tag="big", name="x3T")
                for m in range(TT):
                    tsl = slice(m * P, (m + 1) * P)
                    for j in range(JT):
                        psh = ps_pool.tile([P, P], F32, tag="ps", name="ps")
                        mm_accum(
                            psh[:],
                            [body_sb["w1"][:, k, j * P:(j + 1) * P]
                             for k in range(KT)],
                            [x2T[:, k, tsl] for k in range(KT)],
                        )
                        if not cfg["b1"]:
                            nc.vector.tensor_scalar_max(hT[:, j, tsl], psh[:], 0.0)
                        else:
                            hb = wk.tile([P, P], F32, tag="hb", name="hb")
                            nc.vector.tensor_scalar_add(
                                hb[:], psh[:], biasp_sb[:, 16 + j:16 + j + 1]
                            )
                            nc.vector.tensor_scalar_max(hT[:, j, tsl], hb[:], 0.0)
                    psy = ps_pool.tile([P, D], F32, tag="ps", name="ps")
                    mm_accum(
                        psy[:],
                        [hT[:, j, m * P:(m + 1) * P] for j in range(JT)],
                        [body_sb["w2"][:, j, :] for j in range(JT)],
                    )
                    layer_norm(psy[:], None, x2r[:, m, :], xn3[:, m, :], None)
                    transpose_blocks(x3T, xn3, [(m, b_) for b_ in range(KT)])
                    if after_quarter is not None:
                        after_quarter(m, x3T)
                return xn3

            ag_out = [None] * TT

            def emit_ag(m, x3T):
                ag_in = dr.tile([D, P], BF16, name=f"ag_in{m}", tag=f"ag_in{m}")
                nc.sync.dma_start(
                    ag_in[:, :].rearrange("(k p) t -> p k t", p=P),
                    x3T[:, :, m * P:(m + 1) * P],
                )
                ag_o = dr.tile(
                    [B * D, P], BF16, addr_space="Shared",
                    name=f"ag_out{m}", tag=f"ag_out{m}",
                )
                nc.gpsimd.collective_compute(
                    "AllGather",
                    Alu.bypass,
                    replica_groups=[list(range(N_CORES))],
                    ins=[ag_in[:, :].opt()],
                    outs=[ag_o[:, :].opt()],
                )
                ag_out[m] = ag_o

            def emit_fc(m):
                for b in range(B):
                    xb = fcx.tile([P, KT, P], BF16, tag="xb", name="xb")
                    nc.sync.dma_start(
                        xb[:],
                        ag_out[m][b * D:(b + 1) * D, :]
                        .rearrange("(k p) t -> p k t", p=P),
                    )
                    for v_ in range(NVCH):
                        psl = ps_pool.tile([P, VCH], F32, tag="ps", name="psl")
                        mm_accum(
                            psl[:],
                            [xb[:, k, :] for k in range(KT)],
                            [fcw_sb[:, k, v_ * VCH:(v_ + 1) * VCH]
                             for k in range(KT)],
                        )
                        lsb = wk.tile([P, VCH], BF16, tag="lsb", name="lsb",
                                      bufs=4)
                        if cfg["fcb"]:
                            nc.any.tensor_add(
                                lsb[:], psl[:],
                                gb_sb["fcb"][:, v_ * VCH:(v_ + 1) * VCH],
                            )
                        else:
                            nc.any.tensor_copy(lsb[:], psl[:])
                        nc.sync.dma_start(
                            out_d[b, m * P:(m + 1) * P,
                                  v_ * VCH:(v_ + 1) * VCH],
                            lsb[:],
                        )

            if mode.startswith("body"):
                xn3 = emit_body()
                nc.sync.dma_start(
                    out_d[:, :].rearrange("(m p) d -> p m d", p=P), xn3[:]
                )
            else:
                emit_body(after_quarter=emit_ag)
                if mode == "ag":
                    for m in range(TT):
                        nc.sync.dma_start(
                            out_d[:, m * P:(m + 1) * P], ag_out[m][:, :]
                        )
                else:
                    for m in range(TT):
                        emit_fc(m)

    nc.compile()
    return nc


def _host_prep(inputs):
    """Shared host-side prep: returns (cfg, in_maps, x0_full)."""
    trg = np.asarray(inputs["trg_input"])
    enc = _f32(inputs["encoder_hiddens"])
    src_len = np.asarray(inputs["src_lengths"])
    emb = _f32(inputs["embedding"])
    g = {k: _f32(inputs[k]) for k in (
        "wq1", "bq1", "wk1", "bk1", "wv1", "bv1",
        "wq2", "bq2", "wk2", "bk2", "wv2", "bv2",
        "w1", "b1", "w2", "b2", "fcw", "fcb",
        "g1", "be1", "g2", "be2", "g3", "be3")}

    scale = 1.0 / math.sqrt(float(D))
    pe = _sinusoidal_pe(T, D)
    x0 = emb[trg] + pe[None]                      # [B, T, D] f32

    causal = np.tril(np.ones((T, T), dtype=bool))
    pad = trg != 0                                 # [B, T]
    self_mask = pad[:, None, :] & causal[None]     # [B, T, T]
    self_bias = np.where(self_mask, 0.0, NEG).astype(np.float32)
    exact_mask = bool((~self_mask).all(axis=2).any())

    sidx = np.arange(S)[None, :] < src_len[:, None]   # [B, S]
    cross_bias = np.where(sidx, 0.0, NEG).astype(np.float32)

    fcw_eff = g["g3"][:, None] * g["fcw"]
    fcb_eff = g["be3"] @ g["fcw"] + g["fcb"]

    cfg = {
        "exact_mask": exact_mask,
        "qk1": bool((g["bq1"] == 0.0).all() and (g["bk1"] == 0.0).all()),
        "qk2": bool((g["bq2"] == 0.0).all() and (g["bk2"] == 0.0).all()),
        "b1": bool((g["b1"] != 0.0).any()),
        "gb1": bool((g["g1"] != 1.0).any() or (g["be1"] != 0.0).any()),
        "gb2": bool((g["g2"] != 1.0).any() or (g["be2"] != 0.0).any()),
        "b2": bool((g["b2"] != 0.0).any()),
        "fcb": bool((fcb_eff != 0.0).any()),
    }

    bias_p = np.zeros((P, 32), dtype=np.float32)
    bias_p[:, 0:4] = (g["bq1"] * scale).reshape(KT, P).T
    bias_p[:, 4:8] = g["bk1"].reshape(KT, P).T
    bias_p[:, 8:12] = (g["bq2"] * scale).reshape(KT, P).T
    bias_p[:, 12:16] = g["bk2"].reshape(KT, P).T
    bias_p[:, 16:32] = g["b1"].reshape(JT, P).T
    bias_v = np.stack(
        [np.broadcast_to(g["bv1"], (P, D)), np.broadcast_to(g["bv2"], (P, D))],
        axis=1,
    )

    shared = {
        "wv1": _bf(g["wv1"]), "wv2": _bf(g["wv2"]),
        "w1": _bf(g["w1"]), "w2": _bf(g["w2"]),
        "ident": _bf(np.eye(P, dtype=np.float32)),
        "bias_p": bias_p, "bias_v": _bf(bias_v),
    }
    if cfg["qk1"]:
        shared["wf1"] = _bf((g["wk1"] @ g["wq1"].T) * scale)
    else:
        shared["wq1"] = _bf(g["wq1"] * scale)
        shared["wk1"] = _bf(g["wk1"])
    if cfg["qk2"]:
        shared["wf2"] = _bf((g["wk2"] @ g["wq2"].T) * scale)
    else:
        shared["wq2"] = _bf(g["wq2"] * scale)
        shared["wk2"] = _bf(g["wk2"])
    if cfg["gb1"]:
        shared["gb1_t"] = _bf(np.stack(
            [np.broadcast_to(g["g1"], (P, D)), np.broadcast_to(g["be1"], (P, D))], 1))
    if cfg["gb2"]:
        shared["gb2_t"] = _bf(np.stack(
            [np.broadcast_to(g["g2"], (P, D)), np.broadcast_to(g["be2"], (P, D))], 1))
    if cfg["b2"]:
        shared["b2_t"] = _bf(np.broadcast_to(g["b2"], (P, D)))

    in_maps = []
    for c in range(N_CORES):
        m = dict(shared)
        m["x0"] = _bf(x0[c])
        m["x0T"] = _bf(x0[c].T)
        m["encT"] = _bf(enc[c].T)
        m["self_bias"] = _bf(self_bias[c])
        m["cross_bias"] = _bf(np.broadcast_to(cross_bias[c], (P, S)))
        m["fcw"] = _bf(fcw_eff[:, c * VS:(c + 1) * VS])
        if cfg["exact_mask"]:
            m["self_m01"] = _bf(self_mask[c].astype(np.float32))
        if cfg["fcb"]:
            m["fcb_t"] = _bf(
                np.broadcast_to(fcb_eff[c * VS:(c + 1) * VS], (P, VS)))
        in_maps.append(m)
    return cfg, in_maps, x0


def _filter_in_maps(nc, in_maps):
    """Keep only the dram parameters this graph actually declares."""
    import concourse.mybir as mybir

    declared = set()
    for alloc in nc.m.functions[0].allocations:
        if isinstance(alloc, mybir.MemoryLocationSet) and alloc.kind == "ExternalInput":
            declared.add(alloc.memorylocations[0].name)
    return [{k: v for k, v in m.items() if k in declared} for m in in_maps]


def _run(nc, in_maps):
    global LAST_EXEC_NS, LAST_RESULTS
    from concourse import bass_utils

    # Warm up the PJRT backend with a trivial op first — the bass custom-call
    # as the very first program has been observed to stall device init.
    import jax
    import jax.numpy as jnp

    jnp.add(
        jax.device_put(np.ones((8, 8), np.float32), jax.devices()[0]), 1.0
    ).block_until_ready()

    kwargs = {}
    if TRACE:
        _install_ntff_hook()
        kwargs = {"trace": True}
        if TRACE_DIR:
            kwargs["tmpdir"] = TRACE_DIR
    res = bass_utils.run_bass_kernel_spmd(
        nc, _filter_in_maps(nc, in_maps), core_ids=list(range(N_CORES)), **kwargs
    )
    LAST_EXEC_NS = res.exec_time_ns
    LAST_RESULTS = res
    return res


def kernel(**inputs):
    cfg, in_maps, _ = _host_prep(inputs)
    key = (MODE, DMA_TRANSPOSE) + tuple(sorted(cfg.items()))
    if key not in _cache:
        _cache[key] = _build(cfg, MODE)
    nc = _cache[key]
    res = _run(nc, in_maps)
    if MODE != "full":
        return [np.asarray(res.results[c]["out"]) for c in range(N_CORES)]
    out = np.concatenate(
        [np.asarray(res.results[c]["out"]).astype(np.float32)
         for c in range(N_CORES)],
        axis=2,
    )
    return out
